# revision 1
# baseline (speedup 1.0000x reference)
"""Trainium2 Bass kernel: DCT -> per-image 256-bin histogram -> MLP.

kernel(**inputs) takes the full unsharded inputs (x [32,3,512,512] f32 +
MLP params), shards the 96 images data-parallel across 8 NeuronCores
(12 images each, no collectives), and gathers the [96, 256] f32 output.
"""

import sys

for _p in ("/opt/trn_rl_repo", "/root/.axon_site/_ro/trn_rl_repo"):
    if _p not in sys.path:
        sys.path.insert(0, _p)


import numpy as np
import ml_dtypes
from contextlib import ExitStack

import bass_rust
import concourse.bass as bass
import concourse.mybir as mybir
import concourse.tile as tile
from concourse import bass_isa

FP32 = mybir.dt.float32
FP32R = mybir.dt.float32r
U8 = mybir.dt.uint8
BF16 = mybir.dt.bfloat16
FP8 = mybir.dt.float8e4
AF = mybir.ActivationFunctionType
ALU = mybir.AluOpType

H = W = 512
NBINS = 256
HID = 512
FEAT = 256
HW = H * W


def split_multi_waits(nc, keep=1):
    """Walrus codegen on this toolchain rejects instructions carrying more
    than one semaphore wait. Split excess waits into standalone
    EventSemaphore instructions on the same engine, inserted just before
    the original instruction (same program order => same semantics)."""
    n_split = 0
    for f in nc.m.functions:
        for blk in f.blocks:
            insts = list(blk.instructions)
            new = []
            changed = False
            for inst in insts:
                si = inst.sync_info
                waits = list(si.on_wait) if si is not None else []
                if len(waits) > keep:
                    changed = True
                    for k, w in enumerate(waits[: len(waits) - keep]):
                        ev = bass_rust.InstEventSemaphore(
                            name=f"{inst.name}-sw{k}", ins=[], outs=[]
                        )
                        ev.engine = inst.engine
                        ev.sync_info = bass_rust.SyncInfo(on_wait=[w], on_update=[])
                        new.append(ev)
                        n_split += 1
                    inst.sync_info = bass_rust.SyncInfo(
                        on_wait=waits[len(waits) - keep :],
                        on_update=list(si.on_update),
                    )
                new.append(inst)
            if changed:
                blk.instructions = new
    return n_split


def build_nc(n_img: int, debug_hist: bool = False, split_waits: bool = True, hw_round: bool = True) -> bass.Bass:
    nc = bass.Bass()

    x_in = nc.dram_tensor("x_shard", [n_img, H, W], FP32R, kind="ExternalInput")
    dht_in = nc.dram_tensor("dht", [H, H], FP32R, kind="ExternalInput")
    dwt_in = nc.dram_tensor("dwt", [W, W], FP32R, kind="ExternalInput")
    w1t_in = nc.dram_tensor("w1t", [NBINS, HID], FP32, kind="ExternalInput")
    b1_in = nc.dram_tensor("b1", [HID], FP32, kind="ExternalInput")
    gam_in = nc.dram_tensor("gamma", [HID], FP32, kind="ExternalInput")
    bet_in = nc.dram_tensor("beta", [HID], FP32, kind="ExternalInput")
    w2t_in = nc.dram_tensor("w2t", [HID, FEAT], FP32, kind="ExternalInput")
    b2_in = nc.dram_tensor("b2", [FEAT], FP32, kind="ExternalInput")
    out_dram = nc.dram_tensor("out", [n_img, FEAT], FP32, kind="ExternalOutput")
    hist_dbg = (
        nc.dram_tensor("hist_dbg", [128, 2, n_img], FP32, kind="ExternalOutput")
        if debug_hist
        else None
    )

    with ExitStack() as ctx:
        tc = ctx.enter_context(tile.TileContext(nc))

        const = ctx.enter_context(tc.tile_pool(name="const", bufs=1))
        img_pool = ctx.enter_context(tc.tile_pool(name="img", bufs=3))
        t1t_pool = ctx.enter_context(tc.tile_pool(name="t1t", bufs=2))
        c_pool = ctx.enter_context(tc.tile_pool(name="csb", bufs=2))
        a2_pool = ctx.enter_context(tc.tile_pool(name="a2", bufs=2))
        sq_pool = ctx.enter_context(tc.tile_pool(name="sq", bufs=2))
        u8_pool = ctx.enter_context(tc.tile_pool(name="u8", bufs=3))
        plane_pool = ctx.enter_context(tc.tile_pool(name="planes", bufs=2))
        small_pool = ctx.enter_context(tc.tile_pool(name="small", bufs=3))
        psum_mm = ctx.enter_context(
            tc.tile_pool(name="psum_mm", bufs=2, space="PSUM")
        )
        psum_hist = ctx.enter_context(
            tc.tile_pool(name="psum_hist", bufs=2, space="PSUM")
        )
        psum_mlp = ctx.enter_context(
            tc.tile_pool(name="psum_mlp", bufs=1, space="PSUM")
        )

        # ---- constants into SBUF ----
        dht_sb = const.tile([128, 4, H], FP32R, tag="dht")
        dwt_sb = const.tile([128, 4, W], FP32R, tag="dwt")
        for hc in range(4):
            nc.sync.dma_start(dht_sb[:, hc, :], dht_in[128 * hc : 128 * (hc + 1), :])
            nc.sync.dma_start(dwt_sb[:, hc, :], dwt_in[128 * hc : 128 * (hc + 1), :])
        w1t_sb = const.tile([128, 2, HID], FP32, tag="w1t")
        for kc in range(2):
            nc.sync.dma_start(w1t_sb[:, kc, :], w1t_in[128 * kc : 128 * (kc + 1), :])
        w2t_sb = const.tile([128, 4, FEAT], FP32, tag="w2t")
        for kc in range(4):
            nc.sync.dma_start(w2t_sb[:, kc, :], w2t_in[128 * kc : 128 * (kc + 1), :])
        b1_sb = const.tile([128, 4], FP32, tag="b1")
        gam_sb = const.tile([128, 4], FP32, tag="gam")
        bet_sb = const.tile([128, 4], FP32, tag="bet")
        for mc in range(4):
            nc.sync.dma_start(b1_sb[:, mc : mc + 1], b1_in[128 * mc : 128 * (mc + 1)])
            nc.sync.dma_start(gam_sb[:, mc : mc + 1], gam_in[128 * mc : 128 * (mc + 1)])
            nc.sync.dma_start(bet_sb[:, mc : mc + 1], bet_in[128 * mc : 128 * (mc + 1)])
        b2_sb = const.tile([128, 2], FP32, tag="b2")
        for mc in range(2):
            nc.sync.dma_start(b2_sb[:, mc : mc + 1], b2_in[128 * mc : 128 * (mc + 1)])
        ones_sb = const.tile([128, 1], FP32, tag="ones")
        nc.vector.memset(ones_sb[:], 1.0)
        ones_row = const.tile([1, 128], FP32, tag="ones_row")
        nc.vector.memset(ones_row[:], 1.0)
        negk = const.tile([128, 3], FP32, tag="negk")
        for j, v in enumerate((13.0, 14.0, 15.0)):
            nc.vector.memset(negk[:, j : j + 1], -v)
        one_b = const.tile([128, 1], FP32, tag="one_b")
        nc.vector.memset(one_b[:], 1.0)

        # per-(a,b) counts for all images, bins on partitions: [128, kc, img]
        histT = const.tile([128, 2, n_img], FP32, tag="histT")

        # ---- per-image pipeline ----
        for im in range(n_img):
            img = img_pool.tile([128, 4, W], FP32R, tag="img")
            for hc in range(4):
                nc.sync.dma_start(
                    img[:, hc, :], x_in[im, 128 * hc : 128 * (hc + 1), :]
                )

            # step 1: T1T[w, k] = sum_h img[h, w] * DhT[h, k]
            t1t_sb = t1t_pool.tile([128, 4, H], FP32R, tag="t1t")
            for wc in range(4):
                t1t_ps = psum_mm.tile([128, H], FP32, tag="t1t_ps")
                for hc in range(4):
                    nc.tensor.matmul(
                        t1t_ps[:],
                        img[:, hc, 128 * wc : 128 * (wc + 1)],
                        dht_sb[:, hc, :],
                        start=(hc == 0),
                        stop=(hc == 3),
                    )
                nc.scalar.copy(t1t_sb[:, wc, :], t1t_ps[:])

            # step 2: C[k, l] = sum_w T1T[w, k] * DwT[w, l]
            c_sb = c_pool.tile([128, 4, W], FP32, tag="csb")
            mx4 = small_pool.tile([128, 4], FP32, tag="mx4")
            for kc in range(4):
                c_ps = psum_mm.tile([128, W], FP32, tag="c_ps")
                for wc in range(4):
                    nc.tensor.matmul(
                        c_ps[:],
                        t1t_sb[:, wc, 128 * kc : 128 * (kc + 1)],
                        dwt_sb[:, wc, :],
                        start=(wc == 0),
                        stop=(wc == 3),
                    )
                nc.scalar.copy(c_sb[:, kc, :], c_ps[:])
                nc.vector.tensor_reduce(
                    mx4[:, kc : kc + 1], c_ps[:], axis=mybir.AxisListType.X,
                    op=ALU.max, apply_absolute_value=True,
                )

            # m = max |C| over everything; s = 256/m
            mxp = small_pool.tile([128, 1], FP32, tag="mxp")
            nc.vector.tensor_reduce(
                mxp[:], mx4[:], axis=mybir.AxisListType.X, op=ALU.max
            )
            mxrow = small_pool.tile([1, 128], FP32, tag="mxrow")
            nc.sync.dma_start(mxrow[:], mxp[:])
            s1 = small_pool.tile([1, 1], FP32, tag="s1")
            nc.vector.tensor_reduce(
                s1[:], mxrow[:], axis=mybir.AxisListType.X, op=ALU.max
            )
            nc.vector.tensor_scalar_max(s1[:], s1[:], 1e-12)
            nc.vector.reciprocal(s1[:], s1[:])
            nc.vector.tensor_scalar_mul(s1[:], s1[:], 256.0 * (1.0 - 2.0**-20))
            s_ps = psum_mlp.tile([128, 1], FP32, tag="mlpsmall")
            nc.tensor.matmul(s_ps[:], ones_row[:], s1[:], start=True, stop=True)
            s_sb = small_pool.tile([128, 1], FP32, tag="s")
            nc.vector.tensor_copy(s_sb[:], s_ps[:])

            # idx = floor(|C| * s) as u8 (s keeps max below 256).
            # HW's fp32->u8 cast rounds to nearest, so bias by -0.5 in a
            # second pass; CoreSim truncates, so it skips the bias.
            idx = u8_pool.tile([128, 2048], U8, tag="idx")
            idx_v = idx[:].rearrange("p (c w) -> p c w", c=4)
            if hw_round:
                a2 = a2_pool.tile([128, 4, W], FP32, tag="a2")
                for kc in range(4):
                    nc.scalar.activation(
                        a2[:, kc, :], c_sb[:, kc, :], AF.Abs, scale=s_sb[:]
                    )
                for kc in range(4):
                    nc.scalar.activation(
                        idx_v[:, kc, :], a2[:, kc, :], AF.Copy, bias=-0.5
                    )
            else:
                for kc in range(4):
                    nc.scalar.activation(
                        idx_v[:, kc, :], c_sb[:, kc, :], AF.Abs, scale=s_sb[:]
                    )
            hni = u8_pool.tile([128, 2048], U8, tag="hni")
            lni = u8_pool.tile([128, 2048], U8, tag="lni")
            nc.vector.tensor_scalar(
                hni[:], idx[:], 4, None, op0=ALU.logical_shift_right
            )
            nc.vector.tensor_scalar(lni[:], idx[:], 15, None, op0=ALU.bitwise_and)

            # one-hot planes (interleaved: col = g*16 + value) so each
            # matmul round reads a contiguous [128,128] stationary
            hist_ps = psum_hist.tile([128, 128], FP32, tag="hist")
            for half in range(2):
                u_pl = plane_pool.tile([128, 1024, 16], FP8, tag="u_pl")
                v_pl = plane_pool.tile([128, 1024, 16], FP8, tag="v_pl")
                hsl = hni[:, 1024 * half : 1024 * (half + 1)]
                lsl = lni[:, 1024 * half : 1024 * (half + 1)]
                for a in range(13):
                    nc.vector.tensor_scalar(
                        u_pl[:, :, a], hsl, a, None, op0=ALU.is_equal
                    )
                    nc.vector.tensor_scalar(
                        v_pl[:, :, a], lsl, a, None, op0=ALU.is_equal
                    )
                # planes 13..15 on ACT: Relu(1-(x-a)^2) == [x==a] for ints
                for a in range(13, 16):
                    bk = negk[:, a - 13 : a - 12]
                    squ = sq_pool.tile([128, 1024], FP32, tag="squ")
                    nc.scalar.activation(squ[:], hsl, AF.Square, bias=bk)
                    nc.scalar.activation(
                        u_pl[:, :, a], squ[:], AF.Relu, scale=-1.0, bias=one_b[:]
                    )
                    sqv = sq_pool.tile([128, 1024], FP32, tag="sqv")
                    nc.scalar.activation(sqv[:], lsl, AF.Square, bias=bk)
                    nc.scalar.activation(
                        v_pl[:, :, a], sqv[:], AF.Relu, scale=-1.0, bias=one_b[:]
                    )
                for r in range(128):
                    nc.tensor.matmul(
                        hist_ps[:],
                        u_pl[:, 8 * r : 8 * (r + 1), :],
                        v_pl[:, 8 * r : 8 * (r + 1), :],
                        start=(half == 0 and r == 0),
                        stop=(half == 1 and r == 127),
                    )

            # extract diagonal blocks: cnt(a,b) = sum_i hist_ps[8a+i, 8b+i]
            hist_sb = small_pool.tile([128, 128], FP32, tag="hist_sb")
            nc.scalar.copy(hist_sb[:], hist_ps[:])
            diag8 = small_pool.tile([16, 8, 16], FP32, tag="diag8")
            for j in range(8):
                nc.sync.dma_start(
                    diag8[:, j, :],
                    hist_sb[16 * j : 16 * (j + 1), 16 * j : 16 * (j + 1)],
                )
            cnt16 = small_pool.tile([16, 16], FP32, tag="cnt16")
            nc.vector.tensor_copy(cnt16[:], diag8[:, 0, :])
            for j in range(1, 8):
                nc.vector.tensor_add(cnt16[:], cnt16[:], diag8[:, j, :])

            # scatter counts into histT column (bins on partitions)
            nc.sync.dma_start(histT[:, 0, im : im + 1], cnt16[0:8, :])
            nc.sync.dma_start(histT[:, 1, im : im + 1], cnt16[8:16, :])

        if hist_dbg is not None:
            nc.sync.dma_start(hist_dbg[:], histT[:])

        # ---- MLP over all images ----
        h1_sb = const.tile([128, 4, n_img], FP32, tag="h1")
        h1sq = const.tile([128, 4, n_img], FP32, tag="h1sq")
        for mc in range(4):
            h1_ps = psum_mlp.tile([128, n_img], FP32, tag="mlp_mm")
            for kc in range(2):
                nc.tensor.matmul(
                    h1_ps[:],
                    w1t_sb[:, kc, 128 * mc : 128 * (mc + 1)],
                    histT[:, kc, :],
                    start=(kc == 0),
                    stop=(kc == 1),
                )
            nc.scalar.activation(
                h1_sb[:, mc, :], h1_ps[:], AF.Identity, bias=b1_sb[:, mc : mc + 1]
            )
            nc.scalar.activation(h1sq[:, mc, :], h1_sb[:, mc, :], AF.Square)

        stats_ps = psum_mlp.tile([64, n_img], FP32, tag="mlpsmall")
        musum_ps = stats_ps[0:1, :]
        sqsum_ps = stats_ps[32:33, :]
        for mc in range(4):
            nc.tensor.matmul(
                musum_ps, ones_sb[:], h1_sb[:, mc, :],
                start=(mc == 0), stop=(mc == 3),
            )
        for mc in range(4):
            nc.tensor.matmul(
                sqsum_ps, ones_sb[:], h1sq[:, mc, :],
                start=(mc == 0), stop=(mc == 3),
            )

        stat_row = const.tile([1, 2, n_img], FP32, tag="stat_row")
        mu = stat_row[:, 0, :]
        rstd = stat_row[:, 1, :]
        msq = const.tile([1, n_img], FP32, tag="msq")
        var = const.tile([1, n_img], FP32, tag="var")
        nc.vector.tensor_scalar_mul(mu, musum_ps, 1.0 / HID)
        nc.vector.tensor_scalar_mul(msq[:], sqsum_ps, 1.0 / HID)
        nc.vector.tensor_mul(var[:], mu, mu)
        nc.vector.tensor_sub(var[:], msq[:], var[:])
        nc.vector.tensor_scalar_add(var[:], var[:], 1e-5)
        nc.vector.reciprocal(var[:], var[:])
        nc.scalar.activation(rstd, var[:], AF.Sqrt)

        stat_ps = psum_mlp.tile([128, 2 * n_img], FP32, tag="mlpsmall")
        nc.tensor.matmul(
            stat_ps[:], ones_row[:], stat_row[:, :, :], start=True, stop=True
        )
        stat_f = const.tile([128, 2, n_img], FP32, tag="stat_f")
        nc.vector.tensor_copy(stat_f[:], stat_ps[:])
        mu_f = stat_f[:, 0, :]
        rstd_f = stat_f[:, 1, :]

        r_sb = const.tile([128, 4, n_img], FP32, tag="r_sb")
        for mc in range(4):
            cen = const.tile([128, n_img], FP32, tag="cen")
            nc.vector.tensor_sub(cen[:], h1_sb[:, mc, :], mu_f)
            nc.vector.tensor_mul(cen[:], cen[:], rstd_f)
            nc.scalar.activation(
                r_sb[:, mc, :], cen[:], AF.Relu,
                scale=gam_sb[:, mc : mc + 1], bias=bet_sb[:, mc : mc + 1],
            )

        out_view = out_dram[:].rearrange("n f -> f n")
        for mc in range(2):
            o_ps = psum_mlp.tile([128, n_img], FP32, tag="mlp_mm")
            for kc in range(4):
                nc.tensor.matmul(
                    o_ps[:],
                    w2t_sb[:, kc, 128 * mc : 128 * (mc + 1)],
                    r_sb[:, kc, :],
                    start=(kc == 0),
                    stop=(kc == 3),
                )
            o_sb = const.tile([128, n_img], FP32, tag="o_sb")
            nc.scalar.activation(
                o_sb[:], o_ps[:], AF.Identity, bias=b2_sb[:, mc : mc + 1]
            )
            nc.sync.dma_start(out_view[128 * mc : 128 * (mc + 1), :], o_sb[:])

    if split_waits:
        split_multi_waits(nc)
    return nc


def dct_matrix_np(n):
    k = np.arange(n, dtype=np.float64)[:, None]
    i = np.arange(n, dtype=np.float64)[None, :]
    c = np.cos(np.pi * (2.0 * i + 1.0) * k / (2.0 * n))
    scale = np.where(k == 0, np.sqrt(1.0 / n), np.sqrt(2.0 / n))
    return (scale * c).astype(np.float32)


def host_constants(w1, b1, gamma, beta, w2, b2):
    Dh = dct_matrix_np(H)
    Dw = dct_matrix_np(W)
    return {
        "dht": np.ascontiguousarray(Dh.T),
        "dwt": np.ascontiguousarray(Dw.T),
        "w1t": np.ascontiguousarray((w1 / np.float32(HW)).T.astype(np.float32)),
        "b1": np.ascontiguousarray(b1.astype(np.float32)),
        "gamma": np.ascontiguousarray(gamma.astype(np.float32)),
        "beta": np.ascontiguousarray(beta.astype(np.float32)),
        "w2t": np.ascontiguousarray(w2.T.astype(np.float32)),
        "b2": np.ascontiguousarray(b2.astype(np.float32)),
    }


N_CORES = 8
B, CCH = 32, 3
N_TOTAL = B * CCH  # 96 images
N_PER_CORE = N_TOTAL // N_CORES  # 12

_nc_cache = {}


def _get_nc():
    if "nc" not in _nc_cache:
        _nc_cache["nc"] = build_nc(N_PER_CORE)
    return _nc_cache["nc"]


def kernel(x, w1, b1, gamma, beta, w2, b2):
    from concourse.bass_utils import run_bass_kernel_spmd

    x = np.ascontiguousarray(np.asarray(x, dtype=np.float32))
    consts = host_constants(
        np.asarray(w1, np.float32), np.asarray(b1, np.float32),
        np.asarray(gamma, np.float32), np.asarray(beta, np.float32),
        np.asarray(w2, np.float32), np.asarray(b2, np.float32),
    )
    xf = x.reshape(N_TOTAL, H, W)
    in_maps = []
    for c in range(N_CORES):
        m = dict(consts)
        m["x_shard"] = np.ascontiguousarray(
            xf[c * N_PER_CORE : (c + 1) * N_PER_CORE]
        )
        in_maps.append(m)

    nc = _get_nc()
    res = run_bass_kernel_spmd(nc, in_maps, list(range(N_CORES)))
    outs = [np.asarray(res.results[c]["out"], np.float32) for c in range(N_CORES)]
    return np.concatenate(outs, axis=0)

